# revision 7
# baseline (speedup 1.0000x reference)
"""Bistable recurrent cell layer on 8 Trainium2 NeuronCores.

Data-parallel over batch: each core owns B/8 = 8 batch rows, computes the
three input projections (x@kr, x@kz, x@kh) with the tensor engine, then runs
the T=512 sequential scan with DVE/ACT, all in one NEFF.

Host side: shard + pre-transpose x to [D, B_loc*T] per core (the GEMM needs
d on partitions), gather + re-transpose outputs from [H, B_loc*T].
"""
import os
import sys

for _p in ('/opt/trn_rl_repo', os.path.dirname(os.path.abspath(__file__))):
    if _p not in sys.path:
        sys.path.insert(0, _p)

import numpy as np
from contextlib import ExitStack

import concourse.bass as bass
import concourse.tile as tile
from concourse import bacc, mybir
from concourse.bass_utils import run_bass_kernel_spmd

F32 = mybir.dt.float32
F32R = mybir.dt.float32r
AF = mybir.ActivationFunctionType
OP = mybir.AluOpType

B, T, D, H = 64, 512, 512, 512
NCORES = 8
BL = B // NCORES

last_exec_time_ns = None


def _mm_cast(ap, use_f32r):
    return ap.bitcast(F32R) if use_f32r else ap


def build_body(ctx, tc, aps, cfg):
    """Emit the kernel body.

    aps: dict of DRAM APs. cfg: dict with keys general_m, general_bias,
    general_h0, use_f32r, T, TC, BL.
    """
    nc = tc.nc
    Tt, TC, Bl = cfg['T'], cfg['TC'], cfg['BL']
    nchunk = Tt // TC
    use_f32r = cfg['use_f32r']

    weights = ctx.enter_context(tc.tile_pool(name='weights', bufs=1))
    xt_pool = ctx.enter_context(tc.tile_pool(name='xt', bufs=2))
    prod_pool = ctx.enter_context(tc.tile_pool(name='prod', bufs=2))
    ys_pool = ctx.enter_context(tc.tile_pool(name='ys', bufs=2))
    state = ctx.enter_context(tc.tile_pool(name='state', bufs=1))
    tmp = ctx.enter_context(tc.tile_pool(name='tmp', bufs=3))
    psum_pool = ctx.enter_context(tc.tile_pool(name='psum', bufs=6, space='PSUM'))

    # ---- weights: k order 0=r, 1=z, 2=h ----
    k_sb = []
    for name in ('kr', 'kz', 'kh'):
        t = weights.tile([128, 4, H], F32, tag=name)
        nc.sync.dma_start(t[:], aps[name].rearrange('(dc p) h -> p dc h', p=128))
        k_sb.append(t)

    if cfg['general_bias']:
        b_sb = weights.tile([128, 2, 4], F32, tag='bias')  # [p, (r,z), hb]
        nc.sync.dma_start(b_sb[:, 0, :], aps['br'].rearrange('(hb p) -> p hb', p=128))
        nc.sync.dma_start(b_sb[:, 1, :], aps['bz'].rearrange('(hb p) -> p hb', p=128))
    if cfg['general_m']:
        m_sb = weights.tile([128, 2, 4, Bl], F32, tag='m')  # [p, (r,z), hb, b]
        for i, nm in enumerate(('mr', 'mz')):
            src = aps[nm].rearrange('(hb p) -> p hb', p=128).unsqueeze(2)
            nc.sync.dma_start(m_sb[:, i, :, :], src.broadcast_to([128, 4, Bl]))

    h_last = state.tile([128, 4, Bl], F32, tag='h_last')
    if cfg['general_h0']:
        h0_src = aps['h0'].rearrange('b (hb p) -> p hb b', p=128)
        for hb in range(4):
            nc.sync.dma_start(h_last[:, hb], h0_src[:, hb])
    else:
        nc.vector.memset(h_last[:], 0.0)

    xt_src = aps['xt'].rearrange('(dc p) (b t) -> p dc b t', p=128, b=Bl)
    yt_dst = aps['yt'].rearrange('(hb p) (b t) -> p hb b t', p=128, b=Bl)

    copy_engines = [nc.vector, nc.scalar]

    for ci in range(nchunk):
        t0, t1_ = ci * TC, (ci + 1) * TC

        xt_t = xt_pool.tile([128, 4, Bl, TC], F32, tag='xt')
        for dc in range(4):
            nc.sync.dma_start(xt_t[:, dc], xt_src[:, dc, :, t0:t1_])

        prod = prod_pool.tile([128, 3, 4, Bl, TC], F32, tag='prod')
        icopy = 0
        for ht in range(4):
            for kj in range(3):
                for bh in range(Bl // 4):
                    ps = psum_pool.tile([128, 4 * TC], F32, tag='ps')
                    for dc in range(4):
                        lhsT = k_sb[kj][:, dc, ht * 128:(ht + 1) * 128]
                        rhs = xt_t[:, dc, bh * 4:(bh + 1) * 4, :]
                        nc.tensor.matmul(
                            ps[:], _mm_cast(lhsT, use_f32r),
                            _mm_cast(rhs, use_f32r),
                            start=(dc == 0), stop=(dc == 3))
                    dest = prod[:, kj, ht, bh * 4:(bh + 1) * 4, :]
                    if cfg['general_bias'] and kj < 2:
                        nc.scalar.activation(
                            dest, ps[:].rearrange('p (b t) -> p b t', b=4),
                            AF.Identity, bias=b_sb[:, kj, ht:ht + 1])
                    else:
                        eng = copy_engines[icopy % len(copy_engines)]
                        icopy += 1
                        if eng is nc.scalar:
                            nc.scalar.copy(
                                dest, ps[:].rearrange('p (b t) -> p b t', b=4))
                        else:
                            nc.vector.tensor_copy(
                                dest, ps[:].rearrange('p (b t) -> p b t', b=4))

        # ---- scan over this chunk ----
        ys = ys_pool.tile([128, 4, Bl, TC], F32, tag='ys')
        for tt in range(TC):
            h = h_last[:] if tt == 0 else ys[:, :, :, tt - 1]
            AB = prod[:, 0:2, :, :, tt]     # [128, 2, 4, Bl] (r, z)
            Ct = prod[:, 2, :, :, tt]       # [128, 4, Bl]
            h_b2 = h.unsqueeze(1).broadcast_to([128, 2, 4, Bl])

            szr = tmp.tile([128, 2, 4, Bl], F32, tag='szr')
            if cfg['general_m']:
                hm = tmp.tile([128, 2, 4, Bl], F32, tag='hm')
                nc.vector.tensor_mul(hm[:], h_b2, m_sb[:])
                nc.vector.tensor_add(szr[:], AB, hm[:])
            else:
                nc.vector.tensor_add(szr[:], AB, h_b2)

            t1 = tmp.tile([128, 4, Bl], F32, tag='t1')
            nc.scalar.activation(t1[:], szr[:, 0], AF.Tanh)
            zz = tmp.tile([128, 4, Bl], F32, tag='zz')
            nc.scalar.activation(zz[:], szr[:, 1], AF.Sigmoid)

            rh = tmp.tile([128, 4, Bl], F32, tag='rh')
            nc.vector.scalar_tensor_tensor(
                rh[:], t1[:], 1.0, h, OP.add, OP.mult)
            cc = tmp.tile([128, 4, Bl], F32, tag='cc')
            nc.vector.tensor_add(cc[:], rh[:], Ct)
            gg = tmp.tile([128, 4, Bl], F32, tag='gg')
            nc.scalar.activation(gg[:], cc[:], AF.Tanh)

            ee = tmp.tile([128, 4, Bl], F32, tag='ee')
            nc.vector.tensor_sub(ee[:], h, gg[:])
            ff = tmp.tile([128, 4, Bl], F32, tag='ff')
            nc.vector.tensor_mul(ff[:], zz[:], ee[:])
            nc.vector.tensor_add(ys[:, :, :, tt], ff[:], gg[:])

        nc.gpsimd.tensor_copy(h_last[:], ys[:, :, :, TC - 1])
        for hb in range(4):
            nc.sync.dma_start(yt_dst[:, hb, :, t0:t1_], ys[:, hb])


def build_program(cfg):
    nc = bacc.Bacc('TRN2', target_bir_lowering=False, debug=False)
    Tt, Bl = cfg['T'], cfg['BL']
    aps = {}
    aps['xt'] = nc.dram_tensor('xt', [D, Bl * Tt], F32, kind='ExternalInput').ap()
    for name in ('kr', 'kz', 'kh'):
        aps[name] = nc.dram_tensor(name, [D, H], F32, kind='ExternalInput').ap()
    if cfg['general_m']:
        for name in ('mr', 'mz'):
            aps[name] = nc.dram_tensor(name, [H], F32, kind='ExternalInput').ap()
    if cfg['general_bias']:
        for name in ('br', 'bz'):
            aps[name] = nc.dram_tensor(name, [H], F32, kind='ExternalInput').ap()
    if cfg['general_h0']:
        aps['h0'] = nc.dram_tensor('h0', [Bl, H], F32, kind='ExternalInput').ap()
    aps['yt'] = nc.dram_tensor('yt', [H, Bl * Tt], F32, kind='ExternalOutput').ap()

    with tile.TileContext(nc) as tc, ExitStack() as ctx:
        build_body(ctx, tc, aps, cfg)
    nc.compile()
    return nc


def _install_trace_hook():
    """Register the NTFF profile hook this image's antenv lacks, and neuter
    the cloud artifact upload, so trace=True works locally."""
    import types
    if 'antenv.axon_hooks' not in sys.modules:
        import antenv
        mod = types.ModuleType('antenv.axon_hooks')
        state = {'hook': None}
        mod.set_axon_ntff_profile_hook = lambda h: state.__setitem__('hook', h)
        mod.get_axon_ntff_profile_hook = lambda: state['hook']
        sys.modules['antenv.axon_hooks'] = mod
        antenv.axon_hooks = mod
        from trn_agent_boot.trn_boot import _ntff_profile_via_ctypes
        mod.set_axon_ntff_profile_hook(
            _ntff_profile_via_ctypes('/opt/axon/libaxon_pjrt.so'))
    import concourse.bass_utils as bu
    bu.upload_artifacts = lambda tmpdir: f"local:{tmpdir}"


_programs = {}


def _get_program(key, cfg):
    if key not in _programs:
        _programs[key] = build_program(cfg)
    return _programs[key]


def kernel(x, h0, kz, kr, kh, mz, mr, bz, br):
    global last_exec_time_ns
    x = np.asarray(x, dtype=np.float32)
    h0 = np.asarray(h0, dtype=np.float32)
    kz, kr, kh = (np.asarray(a, dtype=np.float32) for a in (kz, kr, kh))
    mz, mr, bz, br = (np.asarray(a, dtype=np.float32) for a in (mz, mr, bz, br))

    cfg = {
        'T': T, 'TC': 128, 'BL': BL,
        'general_m': not (np.all(mz == 1.0) and np.all(mr == 1.0)),
        'general_bias': not (np.all(bz == 0.0) and np.all(br == 0.0)),
        'general_h0': not np.all(h0 == 0.0),
        'use_f32r': os.environ.get('BRC_F32R', '0') == '1',
    }
    key = tuple(sorted(cfg.items()))
    nc = _get_program(key, cfg)

    in_maps = []
    for c in range(NCORES):
        xi = x[c * BL:(c + 1) * BL]                      # [BL, T, D]
        xt = np.ascontiguousarray(
            xi.transpose(2, 0, 1).reshape(D, BL * T))     # [D, BL*T]
        m = {'xt': xt, 'kr': kr, 'kz': kz, 'kh': kh}
        if cfg['general_m']:
            m['mr'] = mr
            m['mz'] = mz
        if cfg['general_bias']:
            m['br'] = br
            m['bz'] = bz
        if cfg['general_h0']:
            m['h0'] = np.ascontiguousarray(h0[c * BL:(c + 1) * BL])
        in_maps.append(m)

    trace = os.environ.get('BRC_TRACE', '0') == '1'
    if trace:
        _install_trace_hook()
    res = run_bass_kernel_spmd(
        nc, in_maps, core_ids=list(range(NCORES)), trace=trace)
    last_exec_time_ns = res.exec_time_ns
    kernel.last_results = res

    out = np.empty((B, T, H), dtype=np.float32)
    for c in range(NCORES):
        yt = res.results[c]['yt']                         # [H, BL*T]
        out[c * BL:(c + 1) * BL] = (
            yt.reshape(H, BL, T).transpose(1, 2, 0))      # [BL, T, H]
    return out
